# revision 14
# baseline (speedup 1.0000x reference)
"""ChannelWiseFC2d Trainium2 kernel (8 NeuronCores, channel-parallel).

Per (n, c): sort the 1024-vector x[n, c] descending, then
y[n, c, o] = sigmoid(sum_x sorted[x] * W[c, o, x] + b[c, o]).

Sharding: channels 64 -> 8 per core (pure expert parallelism, no
collectives). Per core:
  - bf16 bitonic sort (55 stages) of 2048 rows x 1024 on the DVE.
    Layout trick: the row-block dim t is INNERMOST in SBUF (element i
    of row t at free offset i*TG + t), so every compare-exchange pass
    streams contiguous runs of TG*d elements -- avoiding the ~1.3
    cycle/run AP-step penalty that makes small-d stages 2-2.5x slow
    in the natural layout. Host supplies x pre-interleaved.
  - Two UNEVEN groups (12 + 4 row-blocks): the big group's GEMM
    overlaps the small group's sort, so only the small group's GEMM
    remains as the serial tail.
  - PE transposes sorted 128x128 tiles (x onto partitions) -> lhsT.
  - bf16 matmul vs host-pretransposed W^T tiles, fp32 PSUM accum;
    bias via a K=1 matmul of ones^T @ b; sigmoid on ACT; DMA out.
Host pre/post: x,W,b cast to bf16, W transposed to [c, x, o],
output gathered and transposed to (256, 64, 1024) f32.
"""

import sys

sys.path.insert(0, "/opt/trn_rl_repo")

import numpy as np
import ml_dtypes

import concourse.bass as bass  # noqa: F401  (registers engine classes)
import concourse.mybir as mybir
from concourse import bacc
from concourse.tile import TileContext
from concourse.masks import make_identity
from concourse.bass_utils import run_bass_kernel_spmd

N, C, HW, OUT = 256, 64, 1024, 1024
N_CORES = 8
C_PER = C // N_CORES          # 8 channels per core
ROWS = C_PER * N              # 2048 rows of 1024 per core
NT = ROWS // 128              # 16 row-blocks of 128
GROUP_T = [12, 4]             # row-blocks per group (channel-aligned, uneven)
BF16 = mybir.dt.bfloat16
F32 = mybir.dt.float32
MAX_OP = mybir.AluOpType.max
MIN_OP = mybir.AluOpType.min


def _emit_sort(nc, zbufs, tg, n=HW, k_lo=2, k_hi=HW, cur=0):
    """Bitonic descending sort (levels k_lo..k_hi) of each row of
    zbufs[cur] ([128, n, tg] bf16, row t's element i at free position
    i*tg+t), ping-ponging between zbufs[0]/zbufs[1]. Returns the index
    of the buffer holding the result."""
    k = k_lo
    while k <= k_hi:
        d = k // 2
        while d >= 1:
            src, dst = zbufs[cur], zbufs[1 - cur]
            if k < n:
                a, bsub = n // (2 * k), k // (2 * d)
                pat = "p (a two bsub half d) t -> p two half a bsub (d t)"
                vs = src.rearrange(pat, a=a, two=2, bsub=bsub, half=2, d=d)
                vd = dst.rearrange(pat, a=a, two=2, bsub=bsub, half=2, d=d)
                for two in (0, 1):
                    desc = two == 0
                    nc.vector.tensor_tensor(
                        out=vd[:, two, 0], in0=vs[:, two, 0], in1=vs[:, two, 1],
                        op=MAX_OP if desc else MIN_OP)
                    nc.vector.tensor_tensor(
                        out=vd[:, two, 1], in0=vs[:, two, 0], in1=vs[:, two, 1],
                        op=MIN_OP if desc else MAX_OP)
            else:
                bsub = n // (2 * d)
                pat = "p (bsub half d) t -> p half bsub (d t)"
                vs = src.rearrange(pat, bsub=bsub, half=2, d=d)
                vd = dst.rearrange(pat, bsub=bsub, half=2, d=d)
                nc.vector.tensor_tensor(
                    out=vd[:, 0], in0=vs[:, 0], in1=vs[:, 1], op=MAX_OP)
                nc.vector.tensor_tensor(
                    out=vd[:, 1], in0=vs[:, 0], in1=vs[:, 1], op=MIN_OP)
            cur = 1 - cur
            d //= 2
        k *= 2
    return cur


def _build():
    nc = bacc.Bacc("TRN2", target_bir_lowering=False, debug=False,
                   num_devices=N_CORES)
    # x is one flat [128, HW * NT] bf16 image per partition; group g's
    # block starts at element offset sum(GROUP_T[:g]) * HW and holds
    # [HW, tg] t-innermost data.
    x_ext = nc.declare_dram_parameter("x", [128, HW * NT], BF16, isOutput=False)
    wt_ext = nc.declare_dram_parameter("wt", [C_PER, HW, OUT], BF16,
                                       isOutput=False)
    b_ext = nc.declare_dram_parameter("b", [C_PER, OUT], BF16, isOutput=False)
    out_ext = nc.declare_dram_parameter("out", [C_PER, N, OUT], F32,
                                        isOutput=True)

    w_v = wt_ext.ap().rearrange("c (k p) o -> p c k o", p=128)  # [128, 8, 8, 1024]

    with TileContext(nc) as tc:
        with (
            tc.tile_pool(name="consts", bufs=1) as cpool,
            tc.tile_pool(name="z", bufs=1) as zpool,
            tc.tile_pool(name="st", bufs=1) as stpool,
            tc.tile_pool(name="w", bufs=3) as wpool,
            tc.tile_pool(name="osb", bufs=4) as opool,
            tc.tile_pool(name="tp_psum", bufs=4, space="PSUM") as tppool,
            tc.tile_pool(name="mm_psum", bufs=4, space="PSUM") as mmpool,
        ):
            identity = cpool.tile([128, 128], BF16, tag="ident")
            make_identity(nc, identity)
            ones = cpool.tile([1, 128], BF16, tag="ones")
            nc.gpsimd.memset(ones, 1.0)
            b_sb = cpool.tile([1, C_PER, OUT], BF16, tag="bias")
            nc.sync.dma_start(out=b_sb, in_=b_ext.ap().unsqueeze(0))

            def emit_gemm(g, tg, t_off, zs, copy_engines):
                # Transpose sorted tiles (x onto partitions), then per-channel
                # GEMM + bias + sigmoid + store.
                st = stpool.tile([128, tg, HW // 128, 128], BF16, tag=f"st{g}")
                for i, (t, k) in enumerate(
                        (t, k) for t in range(tg) for k in range(HW // 128)):
                    ps = tppool.tile([128, 128], BF16, tag="tp")
                    nc.tensor.transpose(
                        ps, zs[:, k * 128:(k + 1) * 128, t], identity)
                    copy_engines[i % len(copy_engines)](st[:, t, k, :], ps)
                for cl in range(tg // 2):
                    c = t_off // 2 + cl
                    w_sb = wpool.tile([128, HW // 128, OUT], BF16, tag="w")
                    nc.sync.dma_start(out=w_sb, in_=w_v[:, c])
                    for nt in range(2):
                        t = cl * 2 + nt
                        for oh in range(2):
                            psum = mmpool.tile([128, 512], F32, tag="mm")
                            for k in range(HW // 128):
                                nc.tensor.matmul(
                                    psum,
                                    lhsT=st[:, t, k, :],
                                    rhs=w_sb[:, k, oh * 512:(oh + 1) * 512],
                                    start=(k == 0), stop=False)
                            nc.tensor.matmul(
                                psum, lhsT=ones,
                                rhs=b_sb[:, c, oh * 512:(oh + 1) * 512],
                                start=False, stop=True)
                            o_sb = opool.tile([128, 512], F32, tag="o")
                            nc.scalar.activation(
                                o_sb, psum, mybir.ActivationFunctionType.Sigmoid)
                            nc.sync.dma_start(
                                out=out_ext.ap()[c, nt * 128:(nt + 1) * 128,
                                                 oh * 512:(oh + 1) * 512],
                                in_=o_sb)

            tg0, tg1 = GROUP_T
            zb = []
            for g, tg in enumerate(GROUP_T):
                zb.append([zpool.tile([128, HW, tg], BF16, tag=f"z0g{g}",
                                      name=f"z0g{g}"),
                           zpool.tile([128, HW, tg], BF16, tag=f"z1g{g}",
                                      name=f"z1g{g}")])
            nc.sync.dma_start(
                out=zb[0][0].rearrange("p i t -> p (i t)"),
                in_=x_ext.ap()[:, 0:tg0 * HW])
            nc.sync.dma_start(
                out=zb[1][0].rearrange("p i t -> p (i t)"),
                in_=x_ext.ap()[:, tg0 * HW:NT * HW])
            act_copy = lambda o, i: nc.scalar.copy(o, i)  # noqa: E731
            dve_copy = lambda o, i: nc.vector.tensor_copy(o, i)  # noqa: E731
            cur0 = _emit_sort(nc, zb[0], tg0)
            emit_gemm(0, tg0, 0, zb[0][cur0], [act_copy])
            cur1 = _emit_sort(nc, zb[1], tg1)
            emit_gemm(1, tg1, tg0, zb[1][cur1], [act_copy, dve_copy])
    nc.finalize()
    return nc


_NC = None


def _get_nc():
    global _NC
    if _NC is None:
        _NC = _build()
    return _NC


def kernel(x, W, b):
    x = np.asarray(x)
    W = np.asarray(W)
    b = np.asarray(b)
    xt = x.reshape(N, C, HW).transpose(1, 0, 2)                  # (64, 256, 1024)
    x_f16 = xt.astype(ml_dtypes.bfloat16)
    wt_f16 = W.transpose(0, 2, 1).astype(ml_dtypes.bfloat16)             # (64, x, o)
    b_f16 = b.astype(ml_dtypes.bfloat16)
    in_maps = []
    for m in range(N_CORES):
        xc = x_f16[m * C_PER:(m + 1) * C_PER].reshape(NT, 128, HW)
        # per group: [128, HW, tg] t-innermost, then concat along free dim
        parts = []
        t_off = 0
        for tg in GROUP_T:
            blk = xc[t_off:t_off + tg]                 # [tg, 128, HW]
            parts.append(blk.transpose(1, 2, 0).reshape(128, HW * tg))
            t_off += tg
        in_maps.append({
            "x": np.ascontiguousarray(np.concatenate(parts, axis=1)),
            "wt": np.ascontiguousarray(wt_f16[m * C_PER:(m + 1) * C_PER]),
            "b": np.ascontiguousarray(b_f16[m * C_PER:(m + 1) * C_PER]),
        })
    res = run_bass_kernel_spmd(_get_nc(), in_maps, core_ids=list(range(N_CORES)))
    out = np.concatenate([res.results[m]["out"] for m in range(N_CORES)], axis=0)
    return np.ascontiguousarray(out.transpose(1, 0, 2)).astype(np.float32)


# revision 15
# speedup vs baseline: 1.0101x; 1.0101x over previous
"""ChannelWiseFC2d Trainium2 kernel (8 NeuronCores, channel-parallel).

Per (n, c): sort the 1024-vector x[n, c] descending, then
y[n, c, o] = sigmoid(sum_x sorted[x] * W[c, o, x] + b[c, o]).

Sharding: channels 64 -> 8 per core (pure expert parallelism, no
collectives). Per core:
  - bf16 bitonic sort (55 stages) of 2048 rows x 1024 on the DVE.
    Layout trick: the row-block dim t is INNERMOST in SBUF (element i
    of row t at free offset i*TG + t), so every compare-exchange pass
    streams contiguous runs of TG*d elements -- avoiding the ~1.3
    cycle/run AP-step penalty that makes small-d stages 2-2.5x slow
    in the natural layout. Host supplies x pre-interleaved.
  - Two UNEVEN groups (12 + 4 row-blocks): the big group's GEMM
    overlaps the small group's sort, so only the small group's GEMM
    remains as the serial tail.
  - PE transposes sorted 128x128 tiles (x onto partitions) -> lhsT.
  - bf16 matmul vs host-pretransposed W^T tiles, fp32 PSUM accum;
    bias via a K=1 matmul of ones^T @ b; sigmoid on ACT; DMA out.
Host pre/post: x,W,b cast to bf16, W transposed to [c, x, o],
output gathered and transposed to (256, 64, 1024) f32.
"""

import sys

sys.path.insert(0, "/opt/trn_rl_repo")

import numpy as np
import ml_dtypes

import concourse.bass as bass  # noqa: F401  (registers engine classes)
import concourse.mybir as mybir
from concourse import bacc
from concourse.tile import TileContext
from concourse.masks import make_identity
from concourse.bass_utils import run_bass_kernel_spmd

N, C, HW, OUT = 256, 64, 1024, 1024
N_CORES = 8
C_PER = C // N_CORES          # 8 channels per core
ROWS = C_PER * N              # 2048 rows of 1024 per core
NT = ROWS // 128              # 16 row-blocks of 128
GROUP_T = [12, 4]             # row-blocks per group (channel-aligned, uneven)
BF16 = mybir.dt.bfloat16
F32 = mybir.dt.float32
MAX_OP = mybir.AluOpType.max
MIN_OP = mybir.AluOpType.min


def _emit_sort(nc, zbufs, tg, n=HW, k_lo=2, k_hi=HW, cur=0):
    """Bitonic descending sort (levels k_lo..k_hi) of each row of
    zbufs[cur] ([128, n, tg] bf16, row t's element i at free position
    i*tg+t), ping-ponging between zbufs[0]/zbufs[1]. Returns the index
    of the buffer holding the result."""
    k = k_lo
    while k <= k_hi:
        d = k // 2
        while d >= 1:
            src, dst = zbufs[cur], zbufs[1 - cur]
            if k < n:
                a, bsub = n // (2 * k), k // (2 * d)
                pat = "p (a two bsub half d) t -> p two half a bsub (d t)"
                vs = src.rearrange(pat, a=a, two=2, bsub=bsub, half=2, d=d)
                vd = dst.rearrange(pat, a=a, two=2, bsub=bsub, half=2, d=d)
                for two in (0, 1):
                    desc = two == 0
                    nc.vector.tensor_tensor(
                        out=vd[:, two, 0], in0=vs[:, two, 0], in1=vs[:, two, 1],
                        op=MAX_OP if desc else MIN_OP)
                    nc.vector.tensor_tensor(
                        out=vd[:, two, 1], in0=vs[:, two, 0], in1=vs[:, two, 1],
                        op=MIN_OP if desc else MAX_OP)
            else:
                bsub = n // (2 * d)
                pat = "p (bsub half d) t -> p half bsub (d t)"
                vs = src.rearrange(pat, bsub=bsub, half=2, d=d)
                vd = dst.rearrange(pat, bsub=bsub, half=2, d=d)
                nc.vector.tensor_tensor(
                    out=vd[:, 0], in0=vs[:, 0], in1=vs[:, 1], op=MAX_OP)
                nc.vector.tensor_tensor(
                    out=vd[:, 1], in0=vs[:, 0], in1=vs[:, 1], op=MIN_OP)
            cur = 1 - cur
            d //= 2
        k *= 2
    return cur


def _build():
    nc = bacc.Bacc("TRN2", target_bir_lowering=False, debug=False,
                   num_devices=N_CORES)
    # x is one flat [128, HW * NT] bf16 image per partition; group g's
    # block starts at element offset sum(GROUP_T[:g]) * HW and holds
    # [HW, tg] t-innermost data.
    x_ext = nc.declare_dram_parameter("x", [128, HW * NT], BF16, isOutput=False)
    wt_ext = nc.declare_dram_parameter("wt", [C_PER, HW, OUT], BF16,
                                       isOutput=False)
    b_ext = nc.declare_dram_parameter("b", [C_PER, OUT], BF16, isOutput=False)
    out_ext = nc.declare_dram_parameter("out", [C_PER, N, OUT], F32,
                                        isOutput=True)

    w_v = wt_ext.ap().rearrange("c (k p) o -> p c k o", p=128)  # [128, 8, 8, 1024]

    with TileContext(nc) as tc:
        with (
            tc.tile_pool(name="consts", bufs=1) as cpool,
            tc.tile_pool(name="z", bufs=1) as zpool,
            tc.tile_pool(name="st", bufs=1) as stpool,
            tc.tile_pool(name="w", bufs=3) as wpool,
            tc.tile_pool(name="osb", bufs=4) as opool,
            tc.tile_pool(name="tp_psum", bufs=4, space="PSUM") as tppool,
            tc.tile_pool(name="mm_psum", bufs=4, space="PSUM") as mmpool,
        ):
            identity = cpool.tile([128, 128], BF16, tag="ident")
            make_identity(nc, identity)
            ones = cpool.tile([1, 128], BF16, tag="ones")
            nc.gpsimd.memset(ones, 1.0)
            b_sb = cpool.tile([1, C_PER, OUT], BF16, tag="bias")
            nc.sync.dma_start(out=b_sb, in_=b_ext.ap().unsqueeze(0))

            def emit_gemm(g, tg, t_off, zs, copy_engines):
                # Transpose sorted tiles (x onto partitions), then per-channel
                # GEMM + bias + sigmoid + store.
                st = stpool.tile([128, tg, HW // 128, 128], BF16, tag=f"st{g}")
                for i, (t, k) in enumerate(
                        (t, k) for t in range(tg) for k in range(HW // 128)):
                    ps = tppool.tile([128, 128], BF16, tag="tp")
                    nc.tensor.transpose(
                        ps, zs[:, k * 128:(k + 1) * 128, t], identity)
                    copy_engines[i % len(copy_engines)](st[:, t, k, :], ps)
                for cl in range(tg // 2):
                    c = t_off // 2 + cl
                    w_sb = wpool.tile([128, HW // 128, OUT], BF16, tag="w")
                    nc.sync.dma_start(out=w_sb, in_=w_v[:, c])
                    for nt in range(2):
                        t = cl * 2 + nt
                        for oh in range(2):
                            psum = mmpool.tile([128, 512], F32, tag="mm")
                            for k in range(HW // 128):
                                nc.tensor.matmul(
                                    psum,
                                    lhsT=st[:, t, k, :],
                                    rhs=w_sb[:, k, oh * 512:(oh + 1) * 512],
                                    start=(k == 0), stop=False)
                            nc.tensor.matmul(
                                psum, lhsT=ones,
                                rhs=b_sb[:, c, oh * 512:(oh + 1) * 512],
                                start=False, stop=True)
                            o_sb = opool.tile([128, 512], F32, tag="o")
                            nc.scalar.activation(
                                o_sb, psum, mybir.ActivationFunctionType.Sigmoid)
                            nc.sync.dma_start(
                                out=out_ext.ap()[c, nt * 128:(nt + 1) * 128,
                                                 oh * 512:(oh + 1) * 512],
                                in_=o_sb)

            tg0, tg1 = GROUP_T
            zb = []
            for g, tg in enumerate(GROUP_T):
                zb.append([zpool.tile([128, HW, tg], BF16, tag=f"z0g{g}",
                                      name=f"z0g{g}"),
                           zpool.tile([128, HW, tg], BF16, tag=f"z1g{g}",
                                      name=f"z1g{g}")])
            # Small group's x (1MB) loads first so the DVE can start on its
            # k<=8 levels (~14us) while the big group's x (3MB) streams in;
            # this fills what was ~10us of DVE-idle head.
            nc.sync.dma_start(
                out=zb[1][0].rearrange("p i t -> p (i t)"),
                in_=x_ext.ap()[:, tg0 * HW:NT * HW])
            nc.sync.dma_start(
                out=zb[0][0].rearrange("p i t -> p (i t)"),
                in_=x_ext.ap()[:, 0:tg0 * HW])
            act_copy = lambda o, i: nc.scalar.copy(o, i)  # noqa: E731
            dve_copy = lambda o, i: nc.vector.tensor_copy(o, i)  # noqa: E731
            cur1 = _emit_sort(nc, zb[1], tg1, k_hi=8)
            cur0 = _emit_sort(nc, zb[0], tg0)
            emit_gemm(0, tg0, 0, zb[0][cur0], [act_copy])
            cur1 = _emit_sort(nc, zb[1], tg1, k_lo=16, cur=cur1)
            emit_gemm(1, tg1, tg0, zb[1][cur1], [act_copy, dve_copy])
    nc.finalize()
    return nc


_NC = None


def _get_nc():
    global _NC
    if _NC is None:
        _NC = _build()
    return _NC


def kernel(x, W, b):
    x = np.asarray(x)
    W = np.asarray(W)
    b = np.asarray(b)
    xt = x.reshape(N, C, HW).transpose(1, 0, 2)                  # (64, 256, 1024)
    x_f16 = xt.astype(ml_dtypes.bfloat16)
    wt_f16 = W.transpose(0, 2, 1).astype(ml_dtypes.bfloat16)             # (64, x, o)
    b_f16 = b.astype(ml_dtypes.bfloat16)
    in_maps = []
    for m in range(N_CORES):
        xc = x_f16[m * C_PER:(m + 1) * C_PER].reshape(NT, 128, HW)
        # per group: [128, HW, tg] t-innermost, then concat along free dim
        parts = []
        t_off = 0
        for tg in GROUP_T:
            blk = xc[t_off:t_off + tg]                 # [tg, 128, HW]
            parts.append(blk.transpose(1, 2, 0).reshape(128, HW * tg))
            t_off += tg
        in_maps.append({
            "x": np.ascontiguousarray(np.concatenate(parts, axis=1)),
            "wt": np.ascontiguousarray(wt_f16[m * C_PER:(m + 1) * C_PER]),
            "b": np.ascontiguousarray(b_f16[m * C_PER:(m + 1) * C_PER]),
        })
    res = run_bass_kernel_spmd(_get_nc(), in_maps, core_ids=list(range(N_CORES)))
    out = np.concatenate([res.results[m]["out"] for m in range(N_CORES)], axis=0)
    return np.ascontiguousarray(out.transpose(1, 0, 2)).astype(np.float32)


# revision 16
# speedup vs baseline: 1.0238x; 1.0136x over previous
"""ChannelWiseFC2d Trainium2 kernel (8 NeuronCores, channel-parallel).

Per (n, c): sort the 1024-vector x[n, c] descending, then
y[n, c, o] = sigmoid(sum_x sorted[x] * W[c, o, x] + b[c, o]).

Sharding: channels 64 -> 8 per core (pure expert parallelism, no
collectives). Per core:
  - bf16 bitonic sort (55 stages) of 2048 rows x 1024 on the DVE.
    Layout trick: the row-block dim t is INNERMOST in SBUF (element i
    of row t at free offset i*tg + t), so every compare-exchange pass
    streams contiguous runs of tg*d elements -- avoiding the ~1.3
    cycle/run AP-step penalty that makes small-d stages 2-2.5x slow
    in the natural layout. Host supplies x pre-interleaved.
  - Stages whose AP fits 3 free dims (first stage of each merge level,
    and every k=512 stage) fuse the desc- and asc-direction calls into
    one min + one max call via a diagonal output stride (k+d / k-d).
  - Two UNEVEN groups (12 + 4 row-blocks): the big group's GEMM
    overlaps the small group's sort, so only the small group's GEMM
    remains as the serial tail. The small group's first levels run
    while the big group's x still streams in (head fill), and its
    final merge level is split by i-half so half the tail transposes
    run under the second half's sort.
  - PE transposes sorted 128x128 tiles (x onto partitions) -> lhsT.
  - bf16 matmul vs host-pretransposed W^T tiles, fp32 PSUM accum;
    bias via a K=1 matmul of ones^T @ b; sigmoid on ACT; DMA out.
Host pre/post: x,W,b cast to bf16, W transposed to [c, x, o],
output gathered and transposed to (256, 64, 1024) f32.
"""

import sys

sys.path.insert(0, "/opt/trn_rl_repo")

import numpy as np
import ml_dtypes

import concourse.bass as bass
import concourse.mybir as mybir
from concourse import bacc
from concourse.tile import TileContext
from concourse.masks import make_identity
from concourse.bass_utils import run_bass_kernel_spmd

N, C, HW, OUT = 256, 64, 1024, 1024
N_CORES = 8
C_PER = C // N_CORES          # 8 channels per core
ROWS = C_PER * N              # 2048 rows of 1024 per core
NT = ROWS // 128              # 16 row-blocks of 128
GROUP_T = [12, 4]             # row-blocks per group (channel-aligned, uneven)
BF16 = mybir.dt.bfloat16
F32 = mybir.dt.float32
MAX_OP = mybir.AluOpType.max
MIN_OP = mybir.AluOpType.min


def _stage(nc, src, dst, tg, k, d, n=HW, bslice=None):
    """Emit one bitonic compare-exchange stage (level k, distance d),
    reading src and writing dst ([128, n, tg] bf16, t-innermost)."""
    if k < n:
        a, bsub = n // (2 * k), k // (2 * d)
        if a == 1 or bsub == 1:
            # 3-free-dim case: fuse desc+asc into one max + one min call.
            # Output "diagonal" strides: max outs at desc-A (0) and asc-B
            # (k+d); min outs at desc-B (d) and asc-A (k).
            outer = [2 * k * tg, a] if bsub == 1 else [2 * d * tg, bsub]

            def mk(z, off, two_stride):
                return bass.AP(z.tensor, z.offset + off * tg,
                               [list(z.ap[0]), [two_stride * tg, 2],
                                outer, [1, d * tg]])

            i0, i1 = mk(src, 0, k), mk(src, d, k)
            nc.vector.tensor_tensor(out=mk(dst, 0, k + d), in0=i0, in1=i1,
                                    op=MAX_OP)
            nc.vector.tensor_tensor(out=mk(dst, d, k - d), in0=i0, in1=i1,
                                    op=MIN_OP)
        else:
            pat = "p (a two bsub half d) t -> p two half a bsub (d t)"
            vs = src.rearrange(pat, a=a, two=2, bsub=bsub, half=2, d=d)
            vd = dst.rearrange(pat, a=a, two=2, bsub=bsub, half=2, d=d)
            for two in (0, 1):
                desc = two == 0
                nc.vector.tensor_tensor(
                    out=vd[:, two, 0], in0=vs[:, two, 0], in1=vs[:, two, 1],
                    op=MAX_OP if desc else MIN_OP)
                nc.vector.tensor_tensor(
                    out=vd[:, two, 1], in0=vs[:, two, 0], in1=vs[:, two, 1],
                    op=MIN_OP if desc else MAX_OP)
    else:
        bsub = n // (2 * d)
        pat = "p (bsub half d) t -> p half bsub (d t)"
        vs = src.rearrange(pat, bsub=bsub, half=2, d=d)
        vd = dst.rearrange(pat, bsub=bsub, half=2, d=d)
        sl = slice(None) if bslice is None else bslice
        nc.vector.tensor_tensor(out=vd[:, 0, sl], in0=vs[:, 0, sl],
                                in1=vs[:, 1, sl], op=MAX_OP)
        nc.vector.tensor_tensor(out=vd[:, 1, sl], in0=vs[:, 0, sl],
                                in1=vs[:, 1, sl], op=MIN_OP)


def _emit_sort(nc, zbufs, tg, n=HW, k_lo=2, k_hi=HW, cur=0):
    """Bitonic descending sort (levels k_lo..k_hi); returns the index of
    the buffer holding the result."""
    k = k_lo
    while k <= k_hi:
        d = k // 2
        while d >= 1:
            _stage(nc, zbufs[cur], zbufs[1 - cur], tg, k, d, n)
            cur = 1 - cur
            d //= 2
        k *= 2
    return cur


def _final_level_split(nc, zbufs, tg, cur, half_cb=None, n=HW):
    """The k=n merge level with stages d<=n/4 emitted per i-half, so
    consumers of the first half (half_cb) can run while the second
    half's stages stream on the DVE."""
    _stage(nc, zbufs[cur], zbufs[1 - cur], tg, n, n // 2, n)
    cur = 1 - cur
    ch = cur
    for ihalf in (0, 1):
        ch = cur
        d = n // 4
        while d >= 1:
            nb = (n // 4) // d
            _stage(nc, zbufs[ch], zbufs[1 - ch], tg, n, d, n,
                   bslice=slice(ihalf * nb, (ihalf + 1) * nb))
            ch = 1 - ch
            d //= 2
        if ihalf == 0 and half_cb is not None:
            half_cb(zbufs[ch])
    return ch


def _build():
    nc = bacc.Bacc("TRN2", target_bir_lowering=False, debug=False,
                   num_devices=N_CORES)
    # x is one flat [128, HW * NT] bf16 image per partition; group g's
    # block starts at element offset sum(GROUP_T[:g]) * HW and holds
    # [HW, tg] t-innermost data.
    x_ext = nc.declare_dram_parameter("x", [128, HW * NT], BF16, isOutput=False)
    wt_ext = nc.declare_dram_parameter("wt", [C_PER, HW, OUT], BF16,
                                       isOutput=False)
    b_ext = nc.declare_dram_parameter("b", [C_PER, OUT], BF16, isOutput=False)
    out_ext = nc.declare_dram_parameter("out", [C_PER, N, OUT], F32,
                                        isOutput=True)

    w_v = wt_ext.ap().rearrange("c (k p) o -> p c k o", p=128)  # [128, 8, 8, 1024]

    with TileContext(nc) as tc:
        with (
            tc.tile_pool(name="consts", bufs=1) as cpool,
            tc.tile_pool(name="z", bufs=1) as zpool,
            tc.tile_pool(name="st", bufs=1) as stpool,
            tc.tile_pool(name="w", bufs=3) as wpool,
            tc.tile_pool(name="osb", bufs=4) as opool,
            tc.tile_pool(name="tp_psum", bufs=4, space="PSUM") as tppool,
            tc.tile_pool(name="mm_psum", bufs=4, space="PSUM") as mmpool,
        ):
            identity = cpool.tile([128, 128], BF16, tag="ident")
            make_identity(nc, identity)
            ones = cpool.tile([1, 128], BF16, tag="ones")
            nc.gpsimd.memset(ones, 1.0)
            b_sb = cpool.tile([1, C_PER, OUT], BF16, tag="bias")
            nc.sync.dma_start(out=b_sb, in_=b_ext.ap().unsqueeze(0))

            act_copy = lambda o, i: nc.scalar.copy(o, i)  # noqa: E731
            dve_copy = lambda o, i: nc.vector.tensor_copy(o, i)  # noqa: E731

            def emit_tp(st, zs, tg, krange, engines):
                for t in range(tg):
                    for kk in krange:
                        ps = tppool.tile([128, 128], BF16, tag="tp", name="tp")
                        nc.tensor.transpose(
                            ps, zs[:, kk * 128:(kk + 1) * 128, t], identity)
                        engines[(t * 8 + kk) % len(engines)](st[:, t, kk, :], ps)

            def emit_mm(st, tg, t_off):
                for cl in range(tg // 2):
                    c = t_off // 2 + cl
                    w_sb = wpool.tile([128, HW // 128, OUT], BF16, tag="w",
                                      name="w_sb")
                    nc.sync.dma_start(out=w_sb, in_=w_v[:, c])
                    for nt in range(2):
                        t = cl * 2 + nt
                        for oh in range(2):
                            psum = mmpool.tile([128, 512], F32, tag="mm",
                                               name="mm_ps")
                            for k in range(HW // 128):
                                nc.tensor.matmul(
                                    psum,
                                    lhsT=st[:, t, k, :],
                                    rhs=w_sb[:, k, oh * 512:(oh + 1) * 512],
                                    start=(k == 0), stop=False)
                            nc.tensor.matmul(
                                psum, lhsT=ones,
                                rhs=b_sb[:, c, oh * 512:(oh + 1) * 512],
                                start=False, stop=True)
                            o_sb = opool.tile([128, 512], F32, tag="o",
                                              name="o_sb")
                            nc.scalar.activation(
                                o_sb, psum, mybir.ActivationFunctionType.Sigmoid)
                            nc.sync.dma_start(
                                out=out_ext.ap()[c, nt * 128:(nt + 1) * 128,
                                                 oh * 512:(oh + 1) * 512],
                                in_=o_sb)

            tg0, tg1 = GROUP_T
            zb = []
            for g, tg in enumerate(GROUP_T):
                zb.append([zpool.tile([128, HW, tg], BF16, tag=f"z0g{g}",
                                      name=f"z0g{g}"),
                           zpool.tile([128, HW, tg], BF16, tag=f"z1g{g}",
                                      name=f"z1g{g}")])
            # Small group's x (1MB) loads first so the DVE can start on its
            # k<=8 levels (~14us) while the big group's x (3MB) streams in.
            nc.sync.dma_start(
                out=zb[1][0].rearrange("p i t -> p (i t)"),
                in_=x_ext.ap()[:, tg0 * HW:NT * HW])
            nc.sync.dma_start(
                out=zb[0][0].rearrange("p i t -> p (i t)"),
                in_=x_ext.ap()[:, 0:tg0 * HW])

            cur1 = _emit_sort(nc, zb[1], tg1, k_hi=8)
            cur0 = _emit_sort(nc, zb[0], tg0)
            st0 = stpool.tile([128, tg0, HW // 128, 128], BF16, tag="st0")
            emit_tp(st0, zb[0][cur0], tg0, range(HW // 128), [act_copy])
            emit_mm(st0, tg0, 0)

            cur1 = _emit_sort(nc, zb[1], tg1, k_lo=16, k_hi=HW // 2, cur=cur1)
            st1 = stpool.tile([128, tg1, HW // 128, 128], BF16, tag="st1")
            cur1 = _final_level_split(
                nc, zb[1], tg1, cur1,
                half_cb=lambda zs: emit_tp(st1, zs, tg1, range(4),
                                           [act_copy, dve_copy]))
            emit_tp(st1, zb[1][cur1], tg1, range(4, 8), [act_copy, dve_copy])
            emit_mm(st1, tg1, tg0)
    nc.finalize()
    return nc


_NC = None


def _get_nc():
    global _NC
    if _NC is None:
        _NC = _build()
    return _NC


def kernel(x, W, b):
    x = np.asarray(x)
    W = np.asarray(W)
    b = np.asarray(b)
    xt = x.reshape(N, C, HW).transpose(1, 0, 2)                  # (64, 256, 1024)
    x_bf = xt.astype(ml_dtypes.bfloat16)
    wt_bf = W.transpose(0, 2, 1).astype(ml_dtypes.bfloat16)      # (64, x, o)
    b_bf = b.astype(ml_dtypes.bfloat16)
    in_maps = []
    for m in range(N_CORES):
        xc = x_bf[m * C_PER:(m + 1) * C_PER].reshape(NT, 128, HW)
        # per group: [128, HW, tg] t-innermost, then concat along free dim
        parts = []
        t_off = 0
        for tg in GROUP_T:
            blk = xc[t_off:t_off + tg]                 # [tg, 128, HW]
            parts.append(blk.transpose(1, 2, 0).reshape(128, HW * tg))
            t_off += tg
        in_maps.append({
            "x": np.ascontiguousarray(np.concatenate(parts, axis=1)),
            "wt": np.ascontiguousarray(wt_bf[m * C_PER:(m + 1) * C_PER]),
            "b": np.ascontiguousarray(b_bf[m * C_PER:(m + 1) * C_PER]),
        })
    res = run_bass_kernel_spmd(_get_nc(), in_maps, core_ids=list(range(N_CORES)))
    out = np.concatenate([res.results[m]["out"] for m in range(N_CORES)], axis=0)
    return np.ascontiguousarray(out.transpose(1, 0, 2)).astype(np.float32)


# revision 19
# speedup vs baseline: 1.0296x; 1.0057x over previous
"""ChannelWiseFC2d Trainium2 kernel (8 NeuronCores, channel-parallel).

Per (n, c): sort the 1024-vector x[n, c] descending, then
y[n, c, o] = sigmoid(sum_x sorted[x] * W[c, o, x] + b[c, o]).

Sharding: channels 64 -> 8 per core (pure expert parallelism, no
collectives). Per core:
  - bf16 bitonic sort (55 stages) of 2048 rows x 1024 on the DVE.
    Layout trick: the row-block dim t is INNERMOST in SBUF (element i
    of row t at free offset i*tg + t), so every compare-exchange pass
    streams contiguous runs of tg*d elements -- avoiding the ~1.3
    cycle/run AP-step penalty that makes small-d stages 2-2.5x slow
    in the natural layout. Host supplies x pre-interleaved.
  - Stages whose AP fits 3 free dims (first stage of each merge level,
    and every k=512 stage) fuse the desc- and asc-direction calls into
    one min + one max call via a diagonal output stride (k+d / k-d).
  - Two UNEVEN groups (12 + 4 row-blocks): the big group's GEMM
    overlaps the small group's sort, so only the small group's GEMM
    remains as the serial tail. The small group's first levels run
    while the big group's x still streams in (head fill), and its
    final merge level is split by i-half so half the tail transposes
    run under the second half's sort.
  - PE transposes sorted 128x128 tiles (x onto partitions) -> lhsT.
  - bf16 matmul vs host-pretransposed W^T tiles, fp32 PSUM accum;
    bias via a K=1 matmul of ones^T @ b; sigmoid on ACT; DMA out.
Host pre/post: x,W,b cast to bf16, W transposed to [c, x, o],
output gathered and transposed to (256, 64, 1024) f32.
"""

import sys

sys.path.insert(0, "/opt/trn_rl_repo")

import numpy as np
import ml_dtypes

import concourse.bass as bass
import concourse.mybir as mybir
from concourse import bacc
from concourse.tile import TileContext
from concourse.masks import make_identity
from concourse.bass_utils import run_bass_kernel_spmd

N, C, HW, OUT = 256, 64, 1024, 1024
N_CORES = 8
C_PER = C // N_CORES          # 8 channels per core
ROWS = C_PER * N              # 2048 rows of 1024 per core
NT = ROWS // 128              # 16 row-blocks of 128
GROUP_T = [12, 4]             # row-blocks per group (channel-aligned, uneven)
BF16 = mybir.dt.bfloat16
F32 = mybir.dt.float32
MAX_OP = mybir.AluOpType.max
MIN_OP = mybir.AluOpType.min


def _stage(nc, src, dst, tg, k, d, n=HW, bslice=None):
    """Emit one bitonic compare-exchange stage (level k, distance d),
    reading src and writing dst ([128, n, tg] bf16, t-innermost)."""
    if k < n:
        a, bsub = n // (2 * k), k // (2 * d)
        if a == 1 or bsub == 1:
            # 3-free-dim case: fuse desc+asc into one max + one min call.
            # Output "diagonal" strides: max outs at desc-A (0) and asc-B
            # (k+d); min outs at desc-B (d) and asc-A (k).
            outer = [2 * k * tg, a] if bsub == 1 else [2 * d * tg, bsub]

            def mk(z, off, two_stride):
                return bass.AP(z.tensor, z.offset + off * tg,
                               [list(z.ap[0]), [two_stride * tg, 2],
                                outer, [1, d * tg]])

            i0, i1 = mk(src, 0, k), mk(src, d, k)
            nc.vector.tensor_tensor(out=mk(dst, 0, k + d), in0=i0, in1=i1,
                                    op=MAX_OP)
            nc.vector.tensor_tensor(out=mk(dst, d, k - d), in0=i0, in1=i1,
                                    op=MIN_OP)
        else:
            pat = "p (a two bsub half d) t -> p two half a bsub (d t)"
            vs = src.rearrange(pat, a=a, two=2, bsub=bsub, half=2, d=d)
            vd = dst.rearrange(pat, a=a, two=2, bsub=bsub, half=2, d=d)
            for two in (0, 1):
                desc = two == 0
                nc.vector.tensor_tensor(
                    out=vd[:, two, 0], in0=vs[:, two, 0], in1=vs[:, two, 1],
                    op=MAX_OP if desc else MIN_OP)
                nc.vector.tensor_tensor(
                    out=vd[:, two, 1], in0=vs[:, two, 0], in1=vs[:, two, 1],
                    op=MIN_OP if desc else MAX_OP)
    else:
        bsub = n // (2 * d)
        pat = "p (bsub half d) t -> p half bsub (d t)"
        vs = src.rearrange(pat, bsub=bsub, half=2, d=d)
        vd = dst.rearrange(pat, bsub=bsub, half=2, d=d)
        sl = slice(None) if bslice is None else bslice
        nc.vector.tensor_tensor(out=vd[:, 0, sl], in0=vs[:, 0, sl],
                                in1=vs[:, 1, sl], op=MAX_OP)
        nc.vector.tensor_tensor(out=vd[:, 1, sl], in0=vs[:, 0, sl],
                                in1=vs[:, 1, sl], op=MIN_OP)


def _emit_sort(nc, zbufs, tg, n=HW, k_lo=2, k_hi=HW, cur=0):
    """Bitonic descending sort (levels k_lo..k_hi); returns the index of
    the buffer holding the result."""
    k = k_lo
    while k <= k_hi:
        d = k // 2
        while d >= 1:
            _stage(nc, zbufs[cur], zbufs[1 - cur], tg, k, d, n)
            cur = 1 - cur
            d //= 2
        k *= 2
    return cur


def _final_level_split(nc, zbufs, tg, cur, half_cb=None, n=HW):
    """The k=n merge level with stages d<=n/4 emitted per i-half, so
    consumers of the first half (half_cb) can run while the second
    half's stages stream on the DVE."""
    _stage(nc, zbufs[cur], zbufs[1 - cur], tg, n, n // 2, n)
    cur = 1 - cur
    ch = cur
    for ihalf in (0, 1):
        ch = cur
        d = n // 4
        while d >= 1:
            nb = (n // 4) // d
            _stage(nc, zbufs[ch], zbufs[1 - ch], tg, n, d, n,
                   bslice=slice(ihalf * nb, (ihalf + 1) * nb))
            ch = 1 - ch
            d //= 2
        if ihalf == 0 and half_cb is not None:
            half_cb(zbufs[ch])
    return ch


def _build():
    nc = bacc.Bacc("TRN2", target_bir_lowering=False, debug=False,
                   num_devices=N_CORES)
    # x is one flat [128, HW * NT] bf16 image per partition; group g's
    # block starts at element offset sum(GROUP_T[:g]) * HW and holds
    # [HW, tg] t-innermost data.
    x_ext = nc.declare_dram_parameter("x", [128, HW * NT], BF16, isOutput=False)
    wt_ext = nc.declare_dram_parameter("wt", [C_PER, HW, OUT], BF16,
                                       isOutput=False)
    b_ext = nc.declare_dram_parameter("b", [C_PER, OUT], BF16, isOutput=False)
    out_ext = nc.declare_dram_parameter("out", [C_PER, N, OUT], F32,
                                        isOutput=True)

    w_v = wt_ext.ap().rearrange("c (k p) o -> p c k o", p=128)  # [128, 8, 8, 1024]

    with TileContext(nc) as tc:
        with (
            tc.tile_pool(name="consts", bufs=1) as cpool,
            tc.tile_pool(name="z", bufs=1) as zpool,
            tc.tile_pool(name="st", bufs=1) as stpool,
            tc.tile_pool(name="w", bufs=3) as wpool,
            tc.tile_pool(name="osb", bufs=4) as opool,
            tc.tile_pool(name="tp_psum", bufs=4, space="PSUM") as tppool,
            tc.tile_pool(name="mm_psum", bufs=4, space="PSUM") as mmpool,
        ):
            act_copy = lambda o, i: nc.scalar.copy(o, i)  # noqa: E731
            dve_copy = lambda o, i: nc.vector.tensor_copy(o, i)  # noqa: E731

            def emit_tp(st, zs, tg, krange, engines):
                for t in range(tg):
                    for kk in krange:
                        ps = tppool.tile([128, 128], BF16, tag="tp", name="tp")
                        nc.tensor.transpose(
                            ps, zs[:, kk * 128:(kk + 1) * 128, t], identity)
                        engines[(t * 8 + kk) % len(engines)](st[:, t, kk, :], ps)

            def emit_mm(st, tg, t_off):
                for cl in range(tg // 2):
                    c = t_off // 2 + cl
                    w_sb = wpool.tile([128, HW // 128, OUT], BF16, tag="w",
                                      name="w_sb")
                    nc.sync.dma_start(out=w_sb, in_=w_v[:, c])
                    for nt in range(2):
                        t = cl * 2 + nt
                        for oh in range(2):
                            psum = mmpool.tile([128, 512], F32, tag="mm",
                                               name="mm_ps")
                            for k in range(HW // 128):
                                nc.tensor.matmul(
                                    psum,
                                    lhsT=st[:, t, k, :],
                                    rhs=w_sb[:, k, oh * 512:(oh + 1) * 512],
                                    start=(k == 0), stop=False)
                            nc.tensor.matmul(
                                psum, lhsT=ones,
                                rhs=b_sb[:, c, oh * 512:(oh + 1) * 512],
                                start=False, stop=True)
                            o_sb = opool.tile([128, 512], F32, tag="o",
                                              name="o_sb")
                            nc.scalar.activation(
                                o_sb, psum, mybir.ActivationFunctionType.Sigmoid)
                            nc.sync.dma_start(
                                out=out_ext.ap()[c, nt * 128:(nt + 1) * 128,
                                                 oh * 512:(oh + 1) * 512],
                                in_=o_sb)

            tg0, tg1 = GROUP_T
            zb = []
            for g, tg in enumerate(GROUP_T):
                zb.append([zpool.tile([128, HW, tg], BF16, tag=f"z0g{g}",
                                      name=f"z0g{g}"),
                           zpool.tile([128, HW, tg], BF16, tag=f"z1g{g}",
                                      name=f"z1g{g}")])
            # Small group's x (1MB) loads first so the DVE can start on its
            # k<=8 levels (~14us) while the big group's x (3MB) streams in.
            nc.sync.dma_start(
                out=zb[1][0].rearrange("p i t -> p (i t)"),
                in_=x_ext.ap()[:, tg0 * HW:NT * HW])
            nc.sync.dma_start(
                out=zb[0][0].rearrange("p i t -> p (i t)"),
                in_=x_ext.ap()[:, 0:tg0 * HW])
            # Consts after the x DMAs so they don't delay the head.
            identity = cpool.tile([128, 128], BF16, tag="ident")
            make_identity(nc, identity)
            ones = cpool.tile([1, 128], BF16, tag="ones")
            nc.gpsimd.memset(ones, 1.0)
            b_sb = cpool.tile([1, C_PER, OUT], BF16, tag="bias")
            nc.sync.dma_start(out=b_sb, in_=b_ext.ap().unsqueeze(0))

            cur1 = _emit_sort(nc, zb[1], tg1, k_hi=8)
            cur0 = _emit_sort(nc, zb[0], tg0)
            st0 = stpool.tile([128, tg0, HW // 128, 128], BF16, tag="st0")
            emit_tp(st0, zb[0][cur0], tg0, range(HW // 128), [act_copy])
            emit_mm(st0, tg0, 0)

            cur1 = _emit_sort(nc, zb[1], tg1, k_lo=16, k_hi=HW // 2, cur=cur1)
            st1 = stpool.tile([128, tg1, HW // 128, 128], BF16, tag="st1")
            # Preload g1's weights so its first channel's k0-3 matmuls can
            # run inside the split-final-level window.
            w1 = []
            for cl in range(tg1 // 2):
                w_sb = wpool.tile([128, HW // 128, OUT], BF16, tag="w",
                                  name=f"w_g1_{cl}")
                nc.sync.dma_start(out=w_sb, in_=w_v[:, tg0 // 2 + cl])
                w1.append(w_sb)
            early_ps = {}

            def g1_half0(zs):
                emit_tp(st1, zs, tg1, range(4), [act_copy, dve_copy])
                for nt in range(2):
                    for oh in range(2):
                        psum = mmpool.tile([128, 512], F32, tag="mm",
                                           name="mm_ps_e")
                        for k in range(4):
                            nc.tensor.matmul(
                                psum, lhsT=st1[:, nt, k, :],
                                rhs=w1[0][:, k, oh * 512:(oh + 1) * 512],
                                start=(k == 0), stop=False)
                        early_ps[(nt, oh)] = psum

            cur1 = _final_level_split(nc, zb[1], tg1, cur1, half_cb=g1_half0)
            emit_tp(st1, zb[1][cur1], tg1, range(4, 8), [act_copy, dve_copy])
            c6 = tg0 // 2
            for nt in range(2):
                for oh in range(2):
                    psum = early_ps[(nt, oh)]
                    for k in range(4, 8):
                        nc.tensor.matmul(
                            psum, lhsT=st1[:, nt, k, :],
                            rhs=w1[0][:, k, oh * 512:(oh + 1) * 512],
                            start=False, stop=False)
                    nc.tensor.matmul(
                        psum, lhsT=ones,
                        rhs=b_sb[:, c6, oh * 512:(oh + 1) * 512],
                        start=False, stop=True)
                    o_sb = opool.tile([128, 512], F32, tag="o", name="o_sb")
                    nc.scalar.activation(
                        o_sb, psum, mybir.ActivationFunctionType.Sigmoid)
                    nc.sync.dma_start(
                        out=out_ext.ap()[c6, nt * 128:(nt + 1) * 128,
                                         oh * 512:(oh + 1) * 512],
                        in_=o_sb)
            # channel 7 (t = 2, 3) in full
            for nt in range(2):
                t = 2 + nt
                for oh in range(2):
                    psum = mmpool.tile([128, 512], F32, tag="mm", name="mm_ps")
                    for k in range(HW // 128):
                        nc.tensor.matmul(
                            psum, lhsT=st1[:, t, k, :],
                            rhs=w1[1][:, k, oh * 512:(oh + 1) * 512],
                            start=(k == 0), stop=False)
                    nc.tensor.matmul(
                        psum, lhsT=ones,
                        rhs=b_sb[:, c6 + 1, oh * 512:(oh + 1) * 512],
                        start=False, stop=True)
                    o_sb = opool.tile([128, 512], F32, tag="o", name="o_sb")
                    nc.scalar.activation(
                        o_sb, psum, mybir.ActivationFunctionType.Sigmoid)
                    nc.sync.dma_start(
                        out=out_ext.ap()[c6 + 1, nt * 128:(nt + 1) * 128,
                                         oh * 512:(oh + 1) * 512],
                        in_=o_sb)
    nc.finalize()
    return nc


_NC = None


def _get_nc():
    global _NC
    if _NC is None:
        _NC = _build()
    return _NC


def kernel(x, W, b):
    x = np.asarray(x)
    W = np.asarray(W)
    b = np.asarray(b)
    xt = x.reshape(N, C, HW).transpose(1, 0, 2)                  # (64, 256, 1024)
    x_bf = xt.astype(ml_dtypes.bfloat16)
    wt_bf = W.transpose(0, 2, 1).astype(ml_dtypes.bfloat16)      # (64, x, o)
    b_bf = b.astype(ml_dtypes.bfloat16)
    in_maps = []
    for m in range(N_CORES):
        xc = x_bf[m * C_PER:(m + 1) * C_PER].reshape(NT, 128, HW)
        # per group: [128, HW, tg] t-innermost, then concat along free dim
        parts = []
        t_off = 0
        for tg in GROUP_T:
            blk = xc[t_off:t_off + tg]                 # [tg, 128, HW]
            parts.append(blk.transpose(1, 2, 0).reshape(128, HW * tg))
            t_off += tg
        in_maps.append({
            "x": np.ascontiguousarray(np.concatenate(parts, axis=1)),
            "wt": np.ascontiguousarray(wt_bf[m * C_PER:(m + 1) * C_PER]),
            "b": np.ascontiguousarray(b_bf[m * C_PER:(m + 1) * C_PER]),
        })
    res = run_bass_kernel_spmd(_get_nc(), in_maps, core_ids=list(range(N_CORES)))
    out = np.concatenate([res.results[m]["out"] for m in range(N_CORES)], axis=0)
    return np.ascontiguousarray(out.transpose(1, 0, 2)).astype(np.float32)


# revision 20
# speedup vs baseline: 1.0315x; 1.0018x over previous
"""ChannelWiseFC2d Trainium2 kernel (8 NeuronCores, channel-parallel).

Per (n, c): sort the 1024-vector x[n, c] descending, then
y[n, c, o] = sigmoid(sum_x sorted[x] * W[c, o, x] + b[c, o]).

Sharding: channels 64 -> 8 per core (pure expert parallelism, no
collectives). Per core:
  - bf16 bitonic sort (55 stages) of 2048 rows x 1024 on the DVE.
    Layout trick: the row-block dim t is INNERMOST in SBUF (element i
    of row t at free offset i*tg + t), so every compare-exchange pass
    streams contiguous runs of tg*d elements -- avoiding the ~1.3
    cycle/run AP-step penalty that makes small-d stages 2-2.5x slow
    in the natural layout. Host supplies x pre-interleaved.
  - Stages whose AP fits 3 free dims (first stage of each merge level,
    and every k=512 stage) fuse the desc- and asc-direction calls into
    one min + one max call via a diagonal output stride (k+d / k-d).
  - Two UNEVEN groups (12 + 4 row-blocks): the big group's GEMM
    overlaps the small group's sort, so only the small group's GEMM
    remains as the serial tail. The small group's first levels run
    while the big group's x still streams in (head fill), and its
    final merge level is split by i-half so half the tail transposes
    run under the second half's sort.
  - PE transposes sorted 128x128 tiles (x onto partitions) -> lhsT.
  - bf16 matmul vs host-pretransposed W^T tiles, fp32 PSUM accum;
    bias via a K=1 matmul of ones^T @ b; sigmoid on ACT; DMA out.
Host pre/post: x,W,b cast to bf16, W transposed to [c, x, o],
output gathered and transposed to (256, 64, 1024) f32.
"""

import sys

sys.path.insert(0, "/opt/trn_rl_repo")

import numpy as np
import ml_dtypes

import concourse.bass as bass
import concourse.mybir as mybir
from concourse import bacc
from concourse.tile import TileContext
from concourse.masks import make_identity
from concourse.bass_utils import run_bass_kernel_spmd

N, C, HW, OUT = 256, 64, 1024, 1024
N_CORES = 8
C_PER = C // N_CORES          # 8 channels per core
ROWS = C_PER * N              # 2048 rows of 1024 per core
NT = ROWS // 128              # 16 row-blocks of 128
GROUP_T = [12, 4]             # row-blocks per group (channel-aligned, uneven)
BF16 = mybir.dt.bfloat16
F32 = mybir.dt.float32
MAX_OP = mybir.AluOpType.max
MIN_OP = mybir.AluOpType.min


def _stage(nc, src, dst, tg, k, d, n=HW, bslice=None):
    """Emit one bitonic compare-exchange stage (level k, distance d),
    reading src and writing dst ([128, n, tg] bf16, t-innermost)."""
    if k < n:
        a, bsub = n // (2 * k), k // (2 * d)
        if a == 1 or bsub == 1:
            # 3-free-dim case: fuse desc+asc into one max + one min call.
            # Output "diagonal" strides: max outs at desc-A (0) and asc-B
            # (k+d); min outs at desc-B (d) and asc-A (k).
            outer = [2 * k * tg, a] if bsub == 1 else [2 * d * tg, bsub]

            def mk(z, off, two_stride):
                return bass.AP(z.tensor, z.offset + off * tg,
                               [list(z.ap[0]), [two_stride * tg, 2],
                                outer, [1, d * tg]])

            i0, i1 = mk(src, 0, k), mk(src, d, k)
            nc.vector.tensor_tensor(out=mk(dst, 0, k + d), in0=i0, in1=i1,
                                    op=MAX_OP)
            nc.vector.tensor_tensor(out=mk(dst, d, k - d), in0=i0, in1=i1,
                                    op=MIN_OP)
        else:
            pat = "p (a two bsub half d) t -> p two half a bsub (d t)"
            vs = src.rearrange(pat, a=a, two=2, bsub=bsub, half=2, d=d)
            vd = dst.rearrange(pat, a=a, two=2, bsub=bsub, half=2, d=d)
            for two in (0, 1):
                desc = two == 0
                nc.vector.tensor_tensor(
                    out=vd[:, two, 0], in0=vs[:, two, 0], in1=vs[:, two, 1],
                    op=MAX_OP if desc else MIN_OP)
                nc.vector.tensor_tensor(
                    out=vd[:, two, 1], in0=vs[:, two, 0], in1=vs[:, two, 1],
                    op=MIN_OP if desc else MAX_OP)
    else:
        bsub = n // (2 * d)
        pat = "p (bsub half d) t -> p half bsub (d t)"
        vs = src.rearrange(pat, bsub=bsub, half=2, d=d)
        vd = dst.rearrange(pat, bsub=bsub, half=2, d=d)
        sl = slice(None) if bslice is None else bslice
        nc.vector.tensor_tensor(out=vd[:, 0, sl], in0=vs[:, 0, sl],
                                in1=vs[:, 1, sl], op=MAX_OP)
        nc.vector.tensor_tensor(out=vd[:, 1, sl], in0=vs[:, 0, sl],
                                in1=vs[:, 1, sl], op=MIN_OP)


def _emit_sort(nc, zbufs, tg, n=HW, k_lo=2, k_hi=HW, cur=0):
    """Bitonic descending sort (levels k_lo..k_hi); returns the index of
    the buffer holding the result."""
    k = k_lo
    while k <= k_hi:
        d = k // 2
        while d >= 1:
            _stage(nc, zbufs[cur], zbufs[1 - cur], tg, k, d, n)
            cur = 1 - cur
            d //= 2
        k *= 2
    return cur


def _final_level_split(nc, zbufs, tg, cur, half_cb=None, n=HW):
    """The k=n merge level with stages d<=n/4 emitted per i-half, so
    consumers of the first half (half_cb) can run while the second
    half's stages stream on the DVE."""
    _stage(nc, zbufs[cur], zbufs[1 - cur], tg, n, n // 2, n)
    cur = 1 - cur
    ch = cur
    for ihalf in (0, 1):
        ch = cur
        d = n // 4
        while d >= 1:
            nb = (n // 4) // d
            _stage(nc, zbufs[ch], zbufs[1 - ch], tg, n, d, n,
                   bslice=slice(ihalf * nb, (ihalf + 1) * nb))
            ch = 1 - ch
            d //= 2
        if ihalf == 0 and half_cb is not None:
            half_cb(zbufs[ch])
    return ch


def _build():
    nc = bacc.Bacc("TRN2", target_bir_lowering=False, debug=False,
                   num_devices=N_CORES)
    # x is one flat [128, HW * NT] bf16 image per partition; group g's
    # block starts at element offset sum(GROUP_T[:g]) * HW and holds
    # [HW, tg] t-innermost data.
    x_ext = nc.declare_dram_parameter("x", [128, HW * NT], BF16, isOutput=False)
    wt_ext = nc.declare_dram_parameter("wt", [C_PER, HW, OUT], BF16,
                                       isOutput=False)
    b_ext = nc.declare_dram_parameter("b", [C_PER, OUT], BF16, isOutput=False)
    out_ext = nc.declare_dram_parameter("out", [C_PER, N, OUT], F32,
                                        isOutput=True)

    w_v = wt_ext.ap().rearrange("c (k p) o -> p c k o", p=128)  # [128, 8, 8, 1024]

    with TileContext(nc) as tc:
        with (
            tc.tile_pool(name="consts", bufs=1) as cpool,
            tc.tile_pool(name="z", bufs=1) as zpool,
            tc.tile_pool(name="st", bufs=1) as stpool,
            tc.tile_pool(name="w", bufs=3) as wpool,
            tc.tile_pool(name="osb", bufs=4) as opool,
            tc.tile_pool(name="tp_psum", bufs=4, space="PSUM") as tppool,
            tc.tile_pool(name="mm_psum", bufs=4, space="PSUM") as mmpool,
        ):
            act_copy = lambda o, i: nc.scalar.copy(o, i)  # noqa: E731
            dve_copy = lambda o, i: nc.vector.tensor_copy(o, i)  # noqa: E731

            def emit_tp(st, zs, tg, krange, engines):
                for t in range(tg):
                    for kk in krange:
                        ps = tppool.tile([128, 128], BF16, tag="tp", name="tp")
                        nc.tensor.transpose(
                            ps, zs[:, kk * 128:(kk + 1) * 128, t], identity)
                        engines[(t * 8 + kk) % len(engines)](st[:, t, kk, :], ps)

            def emit_mm(st, tg, t_off):
                for cl in range(tg // 2):
                    c = t_off // 2 + cl
                    w_sb = wpool.tile([128, HW // 128, OUT], BF16, tag="w",
                                      name="w_sb")
                    nc.sync.dma_start(out=w_sb, in_=w_v[:, c])
                    for nt in range(2):
                        t = cl * 2 + nt
                        for oh in range(2):
                            psum = mmpool.tile([128, 512], F32, tag="mm",
                                               name="mm_ps")
                            for k in range(HW // 128):
                                nc.tensor.matmul(
                                    psum,
                                    lhsT=st[:, t, k, :],
                                    rhs=w_sb[:, k, oh * 512:(oh + 1) * 512],
                                    start=(k == 0), stop=False)
                            nc.tensor.matmul(
                                psum, lhsT=ones,
                                rhs=b_sb[:, c, oh * 512:(oh + 1) * 512],
                                start=False, stop=True)
                            o_sb = opool.tile([128, 512], F32, tag="o",
                                              name="o_sb")
                            nc.scalar.activation(
                                o_sb, psum, mybir.ActivationFunctionType.Sigmoid)
                            nc.sync.dma_start(
                                out=out_ext.ap()[c, nt * 128:(nt + 1) * 128,
                                                 oh * 512:(oh + 1) * 512],
                                in_=o_sb)

            tg0, tg1 = GROUP_T
            zb = []
            for g, tg in enumerate(GROUP_T):
                zb.append([zpool.tile([128, HW, tg], BF16, tag=f"z0g{g}",
                                      name=f"z0g{g}"),
                           zpool.tile([128, HW, tg], BF16, tag=f"z1g{g}",
                                      name=f"z1g{g}")])
            # Small group's x (1MB) loads first so the DVE can start on its
            # k<=8 levels (~14us) while the big group's x (3MB) streams in.
            nc.sync.dma_start(
                out=zb[1][0].rearrange("p i t -> p (i t)"),
                in_=x_ext.ap()[:, tg0 * HW:NT * HW])
            nc.sync.dma_start(
                out=zb[0][0].rearrange("p i t -> p (i t)"),
                in_=x_ext.ap()[:, 0:tg0 * HW])
            # Consts after the x DMAs so they don't delay the head.
            identity = cpool.tile([128, 128], BF16, tag="ident")
            make_identity(nc, identity)
            ones = cpool.tile([1, 128], BF16, tag="ones")
            nc.gpsimd.memset(ones, 1.0)
            b_sb = cpool.tile([1, C_PER, OUT], BF16, tag="bias")
            nc.sync.dma_start(out=b_sb, in_=b_ext.ap().unsqueeze(0))

            cur1 = _emit_sort(nc, zb[1], tg1, k_hi=8)
            cur0 = _emit_sort(nc, zb[0], tg0)
            st0 = stpool.tile([128, tg0, HW // 128, 128], BF16, tag="st0")
            emit_tp(st0, zb[0][cur0], tg0, range(HW // 128), [act_copy])
            emit_mm(st0, tg0, 0)

            # Preload g1's weights so its first channel's k0-3 matmuls can
            # run inside the split-final-level window.
            w1 = []
            for cl in range(tg1 // 2):
                w_sb = wpool.tile([128, HW // 128, OUT], BF16, tag="w",
                                  name=f"w_g1_{cl}")
                nc.sync.dma_start(out=w_sb, in_=w_v[:, tg0 // 2 + cl])
                w1.append(w_sb)
            cur1 = _emit_sort(nc, zb[1], tg1, k_lo=16, k_hi=HW // 2, cur=cur1)
            st1 = stpool.tile([128, tg1, HW // 128, 128], BF16, tag="st1")
            early_ps = {}

            def g1_half0(zs):
                # ACT-only copies: a DVE copy here would queue ahead of the
                # second half's sort stages and delay the sort end.
                emit_tp(st1, zs, tg1, range(4), [act_copy])
                for nt in range(2):
                    for oh in range(2):
                        psum = mmpool.tile([128, 512], F32, tag="mm",
                                           name="mm_ps_e")
                        for k in range(4):
                            nc.tensor.matmul(
                                psum, lhsT=st1[:, nt, k, :],
                                rhs=w1[0][:, k, oh * 512:(oh + 1) * 512],
                                start=(k == 0), stop=False)
                        early_ps[(nt, oh)] = psum

            cur1 = _final_level_split(nc, zb[1], tg1, cur1, half_cb=g1_half0)
            emit_tp(st1, zb[1][cur1], tg1, range(4, 8), [act_copy, dve_copy])
            c6 = tg0 // 2
            for nt in range(2):
                for oh in range(2):
                    psum = early_ps[(nt, oh)]
                    for k in range(4, 8):
                        nc.tensor.matmul(
                            psum, lhsT=st1[:, nt, k, :],
                            rhs=w1[0][:, k, oh * 512:(oh + 1) * 512],
                            start=False, stop=False)
                    nc.tensor.matmul(
                        psum, lhsT=ones,
                        rhs=b_sb[:, c6, oh * 512:(oh + 1) * 512],
                        start=False, stop=True)
                    o_sb = opool.tile([128, 512], F32, tag="o", name="o_sb")
                    nc.scalar.activation(
                        o_sb, psum, mybir.ActivationFunctionType.Sigmoid)
                    nc.sync.dma_start(
                        out=out_ext.ap()[c6, nt * 128:(nt + 1) * 128,
                                         oh * 512:(oh + 1) * 512],
                        in_=o_sb)
            # channel 7 (t = 2, 3) in full
            for nt in range(2):
                t = 2 + nt
                for oh in range(2):
                    psum = mmpool.tile([128, 512], F32, tag="mm", name="mm_ps")
                    for k in range(HW // 128):
                        nc.tensor.matmul(
                            psum, lhsT=st1[:, t, k, :],
                            rhs=w1[1][:, k, oh * 512:(oh + 1) * 512],
                            start=(k == 0), stop=False)
                    nc.tensor.matmul(
                        psum, lhsT=ones,
                        rhs=b_sb[:, c6 + 1, oh * 512:(oh + 1) * 512],
                        start=False, stop=True)
                    o_sb = opool.tile([128, 512], F32, tag="o", name="o_sb")
                    nc.scalar.activation(
                        o_sb, psum, mybir.ActivationFunctionType.Sigmoid)
                    nc.sync.dma_start(
                        out=out_ext.ap()[c6 + 1, nt * 128:(nt + 1) * 128,
                                         oh * 512:(oh + 1) * 512],
                        in_=o_sb)
    nc.finalize()
    return nc


_NC = None


def _get_nc():
    global _NC
    if _NC is None:
        _NC = _build()
    return _NC


def kernel(x, W, b):
    x = np.asarray(x)
    W = np.asarray(W)
    b = np.asarray(b)
    xt = x.reshape(N, C, HW).transpose(1, 0, 2)                  # (64, 256, 1024)
    x_bf = xt.astype(ml_dtypes.bfloat16)
    wt_bf = W.transpose(0, 2, 1).astype(ml_dtypes.bfloat16)      # (64, x, o)
    b_bf = b.astype(ml_dtypes.bfloat16)
    in_maps = []
    for m in range(N_CORES):
        xc = x_bf[m * C_PER:(m + 1) * C_PER].reshape(NT, 128, HW)
        # per group: [128, HW, tg] t-innermost, then concat along free dim
        parts = []
        t_off = 0
        for tg in GROUP_T:
            blk = xc[t_off:t_off + tg]                 # [tg, 128, HW]
            parts.append(blk.transpose(1, 2, 0).reshape(128, HW * tg))
            t_off += tg
        in_maps.append({
            "x": np.ascontiguousarray(np.concatenate(parts, axis=1)),
            "wt": np.ascontiguousarray(wt_bf[m * C_PER:(m + 1) * C_PER]),
            "b": np.ascontiguousarray(b_bf[m * C_PER:(m + 1) * C_PER]),
        })
    res = run_bass_kernel_spmd(_get_nc(), in_maps, core_ids=list(range(N_CORES)))
    out = np.concatenate([res.results[m]["out"] for m in range(N_CORES)], axis=0)
    return np.ascontiguousarray(out.transpose(1, 0, 2)).astype(np.float32)


# revision 22
# speedup vs baseline: 1.0316x; 1.0001x over previous
"""ChannelWiseFC2d Trainium2 kernel (8 NeuronCores, channel-parallel).

Per (n, c): sort the 1024-vector x[n, c] descending, then
y[n, c, o] = sigmoid(sum_x sorted[x] * W[c, o, x] + b[c, o]).

Sharding: channels 64 -> 8 per core (pure expert parallelism, no
collectives). Per core:
  - bf16 bitonic sort (55 stages) of 2048 rows x 1024 on the DVE.
    Layout trick: the row-block dim t is INNERMOST in SBUF (element i
    of row t at free offset i*tg + t), so every compare-exchange pass
    streams contiguous runs of tg*d elements -- avoiding the ~1.3
    cycle/run AP-step penalty that makes small-d stages 2-2.5x slow
    in the natural layout. Host supplies x pre-interleaved.
  - Stages whose AP fits 3 free dims (first stage of each merge level,
    and every k=512 stage) fuse the desc- and asc-direction calls into
    one min + one max call via a diagonal output stride (k+d / k-d).
  - Two UNEVEN groups (12 + 4 row-blocks): the big group's GEMM
    overlaps the small group's sort, so only the small group's GEMM
    remains as the serial tail. The small group's first levels run
    while the big group's x still streams in (head fill), and its
    final merge level is split by i-half so half the tail transposes
    run under the second half's sort.
  - PE transposes sorted 128x128 tiles (x onto partitions) -> lhsT.
  - bf16 matmul vs host-pretransposed W^T tiles, fp32 PSUM accum;
    bias via a K=1 matmul of ones^T @ b; sigmoid on ACT; DMA out.
Host pre/post: x,W,b cast to bf16, W transposed to [c, x, o],
output gathered and transposed to (256, 64, 1024) f32.
"""

import sys

sys.path.insert(0, "/opt/trn_rl_repo")

import numpy as np
import ml_dtypes

import concourse.bass as bass
import concourse.mybir as mybir
from concourse import bacc
from concourse.tile import TileContext
from concourse.masks import make_identity
from concourse.bass_utils import run_bass_kernel_spmd

N, C, HW, OUT = 256, 64, 1024, 1024
N_CORES = 8
C_PER = C // N_CORES          # 8 channels per core
ROWS = C_PER * N              # 2048 rows of 1024 per core
NT = ROWS // 128              # 16 row-blocks of 128
GROUP_T = [12, 4]             # row-blocks per group (channel-aligned, uneven)
BF16 = mybir.dt.bfloat16
F32 = mybir.dt.float32
MAX_OP = mybir.AluOpType.max
MIN_OP = mybir.AluOpType.min


def _stage(nc, src, dst, tg, k, d, n=HW, bslice=None):
    """Emit one bitonic compare-exchange stage (level k, distance d),
    reading src and writing dst ([128, n, tg] bf16, t-innermost).
    Returns the first emitted instruction (for dependency pinning)."""
    first = None
    if k < n:
        a, bsub = n // (2 * k), k // (2 * d)
        if a == 1 or bsub == 1:
            # 3-free-dim case: fuse desc+asc into one max + one min call.
            # Output "diagonal" strides: max outs at desc-A (0) and asc-B
            # (k+d); min outs at desc-B (d) and asc-A (k).
            outer = [2 * k * tg, a] if bsub == 1 else [2 * d * tg, bsub]

            def mk(z, off, two_stride):
                return bass.AP(z.tensor, z.offset + off * tg,
                               [list(z.ap[0]), [two_stride * tg, 2],
                                outer, [1, d * tg]])

            i0, i1 = mk(src, 0, k), mk(src, d, k)
            first = nc.vector.tensor_tensor(out=mk(dst, 0, k + d), in0=i0,
                                            in1=i1, op=MAX_OP)
            nc.vector.tensor_tensor(out=mk(dst, d, k - d), in0=i0, in1=i1,
                                    op=MIN_OP)
        else:
            pat = "p (a two bsub half d) t -> p two half a bsub (d t)"
            vs = src.rearrange(pat, a=a, two=2, bsub=bsub, half=2, d=d)
            vd = dst.rearrange(pat, a=a, two=2, bsub=bsub, half=2, d=d)
            for two in (0, 1):
                desc = two == 0
                ins0 = nc.vector.tensor_tensor(
                    out=vd[:, two, 0], in0=vs[:, two, 0], in1=vs[:, two, 1],
                    op=MAX_OP if desc else MIN_OP)
                first = first or ins0
                nc.vector.tensor_tensor(
                    out=vd[:, two, 1], in0=vs[:, two, 0], in1=vs[:, two, 1],
                    op=MIN_OP if desc else MAX_OP)
    else:
        bsub = n // (2 * d)
        pat = "p (bsub half d) t -> p half bsub (d t)"
        vs = src.rearrange(pat, bsub=bsub, half=2, d=d)
        vd = dst.rearrange(pat, bsub=bsub, half=2, d=d)
        sl = slice(None) if bslice is None else bslice
        first = nc.vector.tensor_tensor(out=vd[:, 0, sl], in0=vs[:, 0, sl],
                                        in1=vs[:, 1, sl], op=MAX_OP)
        nc.vector.tensor_tensor(out=vd[:, 1, sl], in0=vs[:, 0, sl],
                                in1=vs[:, 1, sl], op=MIN_OP)
    return first


def _emit_sort(nc, zbufs, tg, n=HW, k_lo=2, k_hi=HW, cur=0):
    """Bitonic descending sort (levels k_lo..k_hi); returns the index of
    the buffer holding the result."""
    k = k_lo
    while k <= k_hi:
        d = k // 2
        while d >= 1:
            _stage(nc, zbufs[cur], zbufs[1 - cur], tg, k, d, n)
            cur = 1 - cur
            d //= 2
        k *= 2
    return cur


def _final_level_split(nc, zbufs, tg, cur, half_cb=None, n=HW):
    """The k=n merge level with stages d<=n/4 emitted per i-half, so
    consumers of the first half (half_cb) can run while the second
    half's stages stream on the DVE."""
    d512_first = _stage(nc, zbufs[cur], zbufs[1 - cur], tg, n, n // 2, n)
    cur = 1 - cur
    ch = cur
    for ihalf in (0, 1):
        ch = cur
        d = n // 4
        while d >= 1:
            nb = (n // 4) // d
            _stage(nc, zbufs[ch], zbufs[1 - ch], tg, n, d, n,
                   bslice=slice(ihalf * nb, (ihalf + 1) * nb))
            ch = 1 - ch
            d //= 2
        if ihalf == 0 and half_cb is not None:
            half_cb(zbufs[ch])
    return ch, d512_first


def _build():
    nc = bacc.Bacc("TRN2", target_bir_lowering=False, debug=False,
                   num_devices=N_CORES)
    # x is one flat [128, HW * NT] bf16 image per partition; group g's
    # block starts at element offset sum(GROUP_T[:g]) * HW and holds
    # [HW, tg] t-innermost data.
    x_ext = nc.declare_dram_parameter("x", [128, HW * NT], BF16, isOutput=False)
    wt_ext = nc.declare_dram_parameter("wt", [C_PER, HW, OUT], BF16,
                                       isOutput=False)
    b_ext = nc.declare_dram_parameter("b", [C_PER, OUT], BF16, isOutput=False)
    out_ext = nc.declare_dram_parameter("out", [C_PER, N, OUT], F32,
                                        isOutput=True)

    w_v = wt_ext.ap().rearrange("c (k p) o -> p c k o", p=128)  # [128, 8, 8, 1024]

    with TileContext(nc) as tc:
        with (
            tc.tile_pool(name="consts", bufs=1) as cpool,
            tc.tile_pool(name="z", bufs=1) as zpool,
            tc.tile_pool(name="st", bufs=1) as stpool,
            tc.tile_pool(name="w", bufs=3) as wpool,
            tc.tile_pool(name="osb", bufs=4) as opool,
            tc.tile_pool(name="tp_psum", bufs=4, space="PSUM") as tppool,
            tc.tile_pool(name="mm_psum", bufs=4, space="PSUM") as mmpool,
        ):
            act_copy = lambda o, i: nc.scalar.copy(o, i)  # noqa: E731
            dve_copy = lambda o, i: nc.vector.tensor_copy(o, i)  # noqa: E731

            def emit_tp(st, zs, tg, krange, engines):
                for t in range(tg):
                    for kk in krange:
                        ps = tppool.tile([128, 128], BF16, tag="tp", name="tp")
                        nc.tensor.transpose(
                            ps, zs[:, kk * 128:(kk + 1) * 128, t], identity)
                        engines[(t * 8 + kk) % len(engines)](st[:, t, kk, :], ps)

            def emit_mm(st, tg, t_off):
                first_mms = []
                for cl in range(tg // 2):
                    c = t_off // 2 + cl
                    w_sb = wpool.tile([128, HW // 128, OUT], BF16, tag="w",
                                      name="w_sb")
                    nc.sync.dma_start(out=w_sb, in_=w_v[:, c])
                    for nt in range(2):
                        t = cl * 2 + nt
                        for oh in range(2):
                            psum = mmpool.tile([128, 512], F32, tag="mm",
                                               name="mm_ps")
                            for k in range(HW // 128):
                                mi = nc.tensor.matmul(
                                    psum,
                                    lhsT=st[:, t, k, :],
                                    rhs=w_sb[:, k, oh * 512:(oh + 1) * 512],
                                    start=(k == 0), stop=False)
                                if k == 0 and nt == 0 and oh == 0:
                                    first_mms.append(mi)
                            nc.tensor.matmul(
                                psum, lhsT=ones,
                                rhs=b_sb[:, c, oh * 512:(oh + 1) * 512],
                                start=False, stop=True)
                            o_sb = opool.tile([128, 512], F32, tag="o",
                                              name="o_sb")
                            nc.scalar.activation(
                                o_sb, psum, mybir.ActivationFunctionType.Sigmoid)
                            nc.sync.dma_start(
                                out=out_ext.ap()[c, nt * 128:(nt + 1) * 128,
                                                 oh * 512:(oh + 1) * 512],
                                in_=o_sb)
                return first_mms

            tg0, tg1 = GROUP_T
            zb = []
            for g, tg in enumerate(GROUP_T):
                zb.append([zpool.tile([128, HW, tg], BF16, tag=f"z0g{g}",
                                      name=f"z0g{g}"),
                           zpool.tile([128, HW, tg], BF16, tag=f"z1g{g}",
                                      name=f"z1g{g}")])
            # Small group's x (1MB) loads first so the DVE can start on its
            # k<=8 levels (~14us) while the big group's x (3MB) streams in.
            nc.sync.dma_start(
                out=zb[1][0].rearrange("p i t -> p (i t)"),
                in_=x_ext.ap()[:, tg0 * HW:NT * HW])
            nc.sync.dma_start(
                out=zb[0][0].rearrange("p i t -> p (i t)"),
                in_=x_ext.ap()[:, 0:tg0 * HW])
            # Consts after the x DMAs so they don't delay the head.
            identity = cpool.tile([128, 128], BF16, tag="ident")
            make_identity(nc, identity)
            ones = cpool.tile([1, 128], BF16, tag="ones")
            nc.gpsimd.memset(ones, 1.0)
            b_sb = cpool.tile([1, C_PER, OUT], BF16, tag="bias")
            nc.sync.dma_start(out=b_sb, in_=b_ext.ap().unsqueeze(0))

            cur1 = _emit_sort(nc, zb[1], tg1, k_hi=8)
            cur0 = _emit_sort(nc, zb[0], tg0)
            st0 = stpool.tile([128, tg0, HW // 128, 128], BF16, tag="st0")
            emit_tp(st0, zb[0][cur0], tg0, range(HW // 128), [act_copy])
            g0_first_mms = emit_mm(st0, tg0, 0)

            # Preload g1's weights so its first channel's k0-3 matmuls can
            # run inside the split-final-level window.
            w1 = []
            for cl in range(tg1 // 2):
                w_sb = wpool.tile([128, HW // 128, OUT], BF16, tag="w",
                                  name=f"w_g1_{cl}")
                nc.sync.dma_start(out=w_sb, in_=w_v[:, tg0 // 2 + cl])
                w1.append(w_sb)
            cur1 = _emit_sort(nc, zb[1], tg1, k_lo=16, k_hi=HW // 2, cur=cur1)
            st1 = stpool.tile([128, tg1, HW // 128, 128], BF16, tag="st1")
            early_ps = {}

            def g1_half0(zs):
                # ACT-only copies: a DVE copy here would queue ahead of the
                # second half's sort stages and delay the sort end.
                emit_tp(st1, zs, tg1, range(4), [act_copy])
                for nt in range(2):
                    for oh in range(2):
                        psum = mmpool.tile([128, 512], F32, tag="mm",
                                           name="mm_ps_e")
                        for k in range(4):
                            nc.tensor.matmul(
                                psum, lhsT=st1[:, nt, k, :],
                                rhs=w1[0][:, k, oh * 512:(oh + 1) * 512],
                                start=(k == 0), stop=False)
                        early_ps[(nt, oh)] = psum

            cur1, d512_inst = _final_level_split(nc, zb[1], tg1, cur1,
                                                 half_cb=g1_half0)
            # Pin g0's last channel's GEMM to g1's final merge level so the
            # PE stays HAM-warm into the tail window instead of idling
            # ~70us and re-throttling to 1.2 GHz.
            from concourse.tile import add_dep_helper
            add_dep_helper(g0_first_mms[-1].ins, d512_inst.ins, sync=True,
                           reason="keep PE warm into g1 tail window")
            emit_tp(st1, zb[1][cur1], tg1, range(4, 8), [act_copy, dve_copy])
            c6 = tg0 // 2
            for nt in range(2):
                for oh in range(2):
                    psum = early_ps[(nt, oh)]
                    for k in range(4, 8):
                        nc.tensor.matmul(
                            psum, lhsT=st1[:, nt, k, :],
                            rhs=w1[0][:, k, oh * 512:(oh + 1) * 512],
                            start=False, stop=False)
                    nc.tensor.matmul(
                        psum, lhsT=ones,
                        rhs=b_sb[:, c6, oh * 512:(oh + 1) * 512],
                        start=False, stop=True)
                    o_sb = opool.tile([128, 512], F32, tag="o", name="o_sb")
                    nc.scalar.activation(
                        o_sb, psum, mybir.ActivationFunctionType.Sigmoid)
                    nc.sync.dma_start(
                        out=out_ext.ap()[c6, nt * 128:(nt + 1) * 128,
                                         oh * 512:(oh + 1) * 512],
                        in_=o_sb)
            # channel 7 (t = 2, 3) in full
            for nt in range(2):
                t = 2 + nt
                for oh in range(2):
                    psum = mmpool.tile([128, 512], F32, tag="mm", name="mm_ps")
                    for k in range(HW // 128):
                        nc.tensor.matmul(
                            psum, lhsT=st1[:, t, k, :],
                            rhs=w1[1][:, k, oh * 512:(oh + 1) * 512],
                            start=(k == 0), stop=False)
                    nc.tensor.matmul(
                        psum, lhsT=ones,
                        rhs=b_sb[:, c6 + 1, oh * 512:(oh + 1) * 512],
                        start=False, stop=True)
                    o_sb = opool.tile([128, 512], F32, tag="o", name="o_sb")
                    nc.scalar.activation(
                        o_sb, psum, mybir.ActivationFunctionType.Sigmoid)
                    nc.sync.dma_start(
                        out=out_ext.ap()[c6 + 1, nt * 128:(nt + 1) * 128,
                                         oh * 512:(oh + 1) * 512],
                        in_=o_sb)
    nc.finalize()
    return nc


_NC = None


def _get_nc():
    global _NC
    if _NC is None:
        _NC = _build()
    return _NC


def kernel(x, W, b):
    x = np.asarray(x)
    W = np.asarray(W)
    b = np.asarray(b)
    xt = x.reshape(N, C, HW).transpose(1, 0, 2)                  # (64, 256, 1024)
    x_bf = xt.astype(ml_dtypes.bfloat16)
    wt_bf = W.transpose(0, 2, 1).astype(ml_dtypes.bfloat16)      # (64, x, o)
    b_bf = b.astype(ml_dtypes.bfloat16)
    in_maps = []
    for m in range(N_CORES):
        xc = x_bf[m * C_PER:(m + 1) * C_PER].reshape(NT, 128, HW)
        # per group: [128, HW, tg] t-innermost, then concat along free dim
        parts = []
        t_off = 0
        for tg in GROUP_T:
            blk = xc[t_off:t_off + tg]                 # [tg, 128, HW]
            parts.append(blk.transpose(1, 2, 0).reshape(128, HW * tg))
            t_off += tg
        in_maps.append({
            "x": np.ascontiguousarray(np.concatenate(parts, axis=1)),
            "wt": np.ascontiguousarray(wt_bf[m * C_PER:(m + 1) * C_PER]),
            "b": np.ascontiguousarray(b_bf[m * C_PER:(m + 1) * C_PER]),
        })
    res = run_bass_kernel_spmd(_get_nc(), in_maps, core_ids=list(range(N_CORES)))
    out = np.concatenate([res.results[m]["out"] for m in range(N_CORES)], axis=0)
    return np.ascontiguousarray(out.transpose(1, 0, 2)).astype(np.float32)
